# revision 16
# baseline (speedup 1.0000x reference)
"""Causal self-attention (dense transformer block) on 8 Trainium2 NeuronCores.

Problem: x[4, 2048, 1024], Wq/Wk/Wv/Wo[1024, 1024] (nn.Linear convention,
y = x @ W.T), 16 heads, head_dim 64, causal softmax attention.

Sharding (hardcoded): core = 2*b + h  where b in 0..3 is the batch index and
h in 0..1 selects heads [8h, 8h+8). Each core computes its batch's attention
for its 8 heads plus the corresponding slice of the output projection,
producing a partial y[2048, 1024]. The host sums the two partials per batch
(the Wo row-split all-reduce done host-side during unsharding).

Kernel (per core, all matmuls in fp32r = tf32 at full PE rate):
  phase 1: PE-transpose Wq/Wk/Wv shards and x; compute Q.T/K.T [d', T]
           (head-pairs packed into 128 partitions) and V [T, d'] augmented
           with a ones column (softmax denominator comes out of the PV
           matmul for free).
  phase 2: causal attention in S.T = K @ Q.T layout ([k, q]): exp on ScalarE
           straight out of PSUM, PV accumulates O.T (+denominator row),
           normalization via reciprocal + K=1 outer-product broadcast.
  phase 3: output projection y += O_h.T.T @ WoT_h per head, streamed to DRAM.
"""
import os
import numpy as np

B, T, C = 4, 2048, 1024
N_HEADS, HEAD_DIM = 16, 64
H = 8              # heads per core
DSH = 512          # feature shard per core
P = 128
TB = 256           # phase-1 t-block
NTB = T // TB      # 8
CC = C // P        # 8 contraction chunks
QT = 512           # attention q tile
NQ = T // QT       # 4
SCALE = 1.0 / 8.0  # 1/sqrt(head_dim)

_CACHE = {}


def _build(reps=1, parts="all"):
    import concourse.bass as bass
    import concourse.mybir as mybir
    import concourse.tile as tile
    from concourse import bacc
    from concourse.masks import make_identity

    f32 = mybir.dt.float32
    f32r = mybir.dt.float32r
    Exp = mybir.ActivationFunctionType.Exp

    nc = bacc.Bacc("TRN2", target_bir_lowering=False, debug=False)

    x_d = nc.dram_tensor("x", [T, C], f32, kind="ExternalInput").ap()
    wq_d = nc.dram_tensor("Wq", [DSH, C], f32, kind="ExternalInput").ap()
    wk_d = nc.dram_tensor("Wk", [DSH, C], f32, kind="ExternalInput").ap()
    wv_d = nc.dram_tensor("Wv", [DSH, C], f32, kind="ExternalInput").ap()
    wo_d = nc.dram_tensor("Wo", [C, DSH], f32, kind="ExternalInput").ap()
    y_d = nc.dram_tensor("y", [T, C], f32, kind="ExternalOutput").ap()

    with tile.TileContext(nc) as tc:
      for _rep in range(reps):
        with tc.tile_pool(name="persist", bufs=1) as pp:
            # constants: identity (PE transpose), causal mask, ones
            consts = pp.tile([P, P + 256], f32)
            ident = consts[:, 0:P]
            make_identity(nc, ident)
            # additive causal mask in [k, q] layout:
            #   cols 0..127   : all -1e30 (fully invalid columns, j==3 blocks)
            #   cols 128..255 : 0 where q_local >= k else -1e30 (the triangle)
            cmask = consts[:, P:P + 256]
            nc.gpsimd.memset(cmask[:, 0:P], -1e30)
            nc.gpsimd.memset(cmask[:, P:2 * P], 0.0)
            nc.gpsimd.affine_select(
                out=cmask[:, P:2 * P], in_=cmask[:, P:2 * P],
                compare_op=mybir.AluOpType.is_ge, fill=-1e30,
                base=0, pattern=[[1, P]], channel_multiplier=-1,
            )
            ones_f32 = pp.tile([P, P], f32)
            nc.vector.memset(ones_f32, 1.0)
            ones_r = pp.tile([P, 64], f32r)
            nc.vector.tensor_copy(ones_r, ones_f32[:, 0:64])

            # persistent activations
            # qt/kt: [128, 4, T]; partition = (h%2)*64 + dh, dim1 = h//2
            qt_sb = pp.tile([P, 4, T], f32r)
            kt_sb = pp.tile([P, 4, T], f32r)
            # vag: [128, T/128, h, 65]; [.., 0:64] = V, [.., 64] = ones
            vag_sb = pp.tile([P, T // P, H, 65], f32r)
            nc.vector.tensor_copy(
                vag_sb[:, :, :, 64],
                ones_f32.rearrange("p (a b) -> p a b", a=T // P)[:, :, 0:H],
            )

            if parts == "att":
                # timing-only variant: touch the activation tensors so Tile
                # sees a writer; contents are garbage
                nc.vector.memset(qt_sb.bitcast(f32)[:, :, 0:64], 0.125)
                nc.vector.memset(kt_sb.bitcast(f32)[:, :, 0:64], 0.125)
                nc.vector.memset(vag_sb.bitcast(f32)[:, 0, :, :], 0.125)

            # ---------------- phase 1: weights + projections ----------------
            if parts in ("all", "p1"):
             with tc.tile_pool(name="wts", bufs=1) as wpool, \
                 tc.tile_pool(name="stage", bufs=2) as spool, \
                 tc.tile_pool(name="pproj", bufs=4, space="PSUM") as ppsum, \
                 tc.tile_pool(name="ptr", bufs=2, space="PSUM") as tpsum:
                # W transposes: wt[c partition, c-chunk, d'] = W.T
                wts = []
                for wname, w_dram in (("wq", wq_d), ("wk", wk_d), ("wv", wv_d)):
                    wt = wpool.tile([P, CC, DSH], f32r, name=f"{wname}t")
                    wts.append(wt)
                    for half in range(2):
                        wnat = spool.tile([P, 2, C], f32, tag="wnat", bufs=1)
                        nc.sync.dma_start(
                            wnat,
                            w_dram[half * 256:(half + 1) * 256]
                            .rearrange("(o p) c -> p o c", p=P),
                        )
                        for i in range(2):
                            dt_ = half * 2 + i
                            for ccg in range(2):
                                ps = tpsum.tile([P, 4, P], f32, tag="tp")
                                for j in range(4):
                                    cc = ccg * 4 + j
                                    nc.tensor.transpose(
                                        ps[:, j], wnat[:, i, cc * P:(cc + 1) * P],
                                        ident)
                                nc.scalar.copy(
                                    wt[:, ccg * 4:(ccg + 1) * 4,
                                       dt_ * P:(dt_ + 1) * P], ps)
                wqt, wkt, wvt = wts

                # x stream: transpose + projections per t-block of 512
                for tb in range(T // 512):
                    xt = spool.tile([P, CC, 512], f32r, tag="xt")
                    for xh in range(2):  # halves of 256 rows
                        xnat = spool.tile([P, 2, C], f32, tag="xnat",
                                          name=f"xnat{xh}")
                        for i in range(2):
                            r0 = tb * 512 + xh * 256 + i * P
                            nc.sync.dma_start(xnat[:, i], x_d[r0:r0 + P, :])
                        for tt in range(2):
                            for ccg in range(2):
                                ps = tpsum.tile([P, 4, P], f32, tag="tp")
                                for j in range(4):
                                    cc = ccg * 4 + j
                                    nc.tensor.transpose(
                                        ps[:, j],
                                        xnat[:, tt, cc * P:(cc + 1) * P],
                                        ident)
                                nc.vector.tensor_copy(
                                    xt[:, ccg * 4:(ccg + 1) * 4,
                                       (xh * 2 + tt) * P:(xh * 2 + tt + 1) * P],
                                    ps)
                    # Q.T / K.T projections: psum [d' 128, t 512]
                    for wt, out_sb in ((wqt, qt_sb), (wkt, kt_sb)):
                        for dt_ in range(4):
                            ps = ppsum.tile([P, 512], f32, tag="pj")
                            for cc in range(CC):
                                nc.tensor.matmul(
                                    ps, wt[:, cc, dt_ * P:(dt_ + 1) * P],
                                    xt[:, cc],
                                    start=(cc == 0), stop=(cc == CC - 1))
                            nc.vector.tensor_copy(
                                out_sb[:, dt_, tb * 512:(tb + 1) * 512], ps)
                    # V projection: psum [t 128, d' 512]
                    for tt in range(4):
                        ps = ppsum.tile([P, DSH], f32, tag="pj")
                        for cc in range(CC):
                            nc.tensor.matmul(
                                ps, xt[:, cc, tt * P:(tt + 1) * P], wvt[:, cc],
                                start=(cc == 0), stop=(cc == CC - 1))
                        kt_idx = tb * 4 + tt
                        nc.vector.tensor_copy(
                            vag_sb[:, kt_idx, :, 0:64],
                            ps.rearrange("p (h d) -> p h d", h=H))

            # ---------------- phases 2+3: attention + output proj ----------------
            if parts in ("all", "att"):
             with tc.tile_pool(name="att", bufs=1) as apool, \
                 tc.tile_pool(name="wk2", bufs=2) as wkpool, \
                 tc.tile_pool(name="ps_s", bufs=2, space="PSUM") as spsum, \
                 tc.tile_pool(name="ps_o", bufs=2, space="PSUM") as opsum, \
                 tc.tile_pool(name="ps_m", bufs=2, space="PSUM") as mpsum:
                # Wo transpose: wot[c' (64), h, d] = Wo[:, shard].T per head
                wot = apool.tile([64, H, C], f32r)
                for o in range(CC):
                    wonat = wkpool.tile([P, DSH], f32, tag="wonat")
                    nc.sync.dma_start(wonat, wo_d[o * P:(o + 1) * P, :])
                    for h_ in range(H):
                        tp = mpsum.tile([64, P], f32, tag="mp")
                        nc.tensor.transpose(
                            tp, wonat[:, h_ * 64:(h_ + 1) * 64], ident)
                        nc.scalar.copy(wot[:, h_, o * P:(o + 1) * P], tp)

                for qi in range(NQ):
                    q_lo = qi * QT
                    n_kt = (q_lo + QT) // P
                    ot = wkpool.tile([64, H, QT], f32r, tag="ot", bufs=1)
                    for pair in range(4):
                        o_ps = [opsum.tile([65, QT], f32, tag="op",
                                           name=f"ops{pair}_{sub}")
                                for sub in range(2)]
                        # groups of 2 k-tiles: one exp instruction per group
                        for g in range(0, n_kt, 2):
                            cols = []
                            for ki in range(2):
                                kt = g + ki
                                tri_lo = kt * P - q_lo
                                col_ex = max(0, tri_lo)
                                cols.append((min(col_ex, QT - 256), col_ex,
                                             tri_lo))
                            gcol = cols[0][0]  # widest (earlier kt) range
                            ess = []
                            for sub in range(2):
                                h_ = pair * 2 + sub
                                pb = sub * 64
                                s2 = spsum.tile([P, 2, QT], f32, tag="sp",
                                                name=f"sps{sub}")
                                for ki in range(2):
                                    kt = g + ki
                                    col_lo, col_ex, tri_lo = cols[ki]
                                    nc.tensor.matmul(
                                        s2[:, ki, col_lo:QT],
                                        kt_sb[pb:pb + 64, pair,
                                              kt * P:(kt + 1) * P],
                                        qt_sb[pb:pb + 64, pair,
                                              q_lo + col_lo:q_lo + QT],
                                        start=True, stop=True)
                                    if tri_lo >= 0:
                                        w = col_ex + P - col_lo
                                        nc.vector.tensor_add(
                                            s2[:, ki, col_lo:col_ex + P],
                                            s2[:, ki, col_lo:col_ex + P],
                                            cmask[:, 256 - w:256])
                                es = wkpool.tile([P, 2, QT], f32r, tag="es",
                                                 bufs=3, name=f"es{sub}")
                                nc.scalar.activation(
                                    es[:, :, gcol:QT], s2[:, :, gcol:QT],
                                    Exp, scale=SCALE)
                                ess.append(es)
                            for sub in range(2):
                                h_ = pair * 2 + sub
                                for ki in range(2):
                                    kt = g + ki
                                    col_lo = cols[ki][0]
                                    nc.tensor.matmul(
                                        o_ps[sub][:, col_lo:QT],
                                        vag_sb[:, kt, h_, :],
                                        ess[sub][:, ki, col_lo:QT],
                                        start=(kt == 0), stop=(kt == n_kt - 1))
                        for sub in range(2):
                            h_ = pair * 2 + sub
                            o_sb = wkpool.tile([65, QT], f32, tag="ob", bufs=3,
                                               name=f"osb{sub}")
                            nc.vector.tensor_copy(o_sb, o_ps[sub])
                            rec = wkpool.tile([65, QT], f32r, tag="rb", bufs=2,
                                              name=f"rec{sub}")
                            with nc.allow_low_precision(
                                    reason="softmax denominator in tf32"):
                                nc.vector.reciprocal(
                                    rec[64:65, :], o_sb[64:65, :])
                            bc_ps = mpsum.tile([64, QT], f32, tag="mp")
                            nc.tensor.matmul(
                                bc_ps, ones_r[64:65, 0:64], rec[64:65, :],
                                start=True, stop=True)
                            nc.vector.tensor_mul(
                                ot[:, h_, :], o_sb[0:64, :], bc_ps)
                    # output projection for this q tile
                    for tt in range(QT // P):
                        for nn in range(2):
                            y_ps = mpsum.tile([P, 512], f32, tag="mp")
                            for h_ in range(H):
                                nc.tensor.matmul(
                                    y_ps, ot[:, h_, tt * P:(tt + 1) * P],
                                    wot[:, h_, nn * 512:(nn + 1) * 512],
                                    start=(h_ == 0), stop=(h_ == H - 1))
                            y_sb = wkpool.tile([P, 512], f32, tag="ysb", bufs=2)
                            nc.vector.tensor_copy(y_sb, y_ps)
                            nc.sync.dma_start(
                                y_d[q_lo + tt * P:q_lo + (tt + 1) * P,
                                    nn * 512:(nn + 1) * 512], y_sb)

    nc.compile()
    return nc


def _get_runner(reps=1, parts="all"):
    """Build the Bass program once and wrap it in a cached 8-core jitted fn."""
    key = f"runner{reps}_{parts}"
    if key in _CACHE:
        return _CACHE[key]
    import jax
    from jax.experimental.shard_map import shard_map
    from jax.sharding import Mesh, PartitionSpec
    import concourse.mybir as mybir
    from concourse import bass2jax

    nc = _build(reps, parts)
    bass2jax.install_neuronx_cc_hook()

    partition_name = (nc.partition_id_tensor.name
                      if nc.partition_id_tensor else None)
    in_names, out_names, out_avals, zero_shapes = [], [], [], []
    for alloc in nc.m.functions[0].allocations:
        if not isinstance(alloc, mybir.MemoryLocationSet):
            continue
        name = alloc.memorylocations[0].name
        if alloc.kind == "ExternalInput":
            if name != partition_name:
                in_names.append(name)
        elif alloc.kind == "ExternalOutput":
            out_names.append(name)
            shape = tuple(alloc.tensor_shape)
            dtype = mybir.dt.np(alloc.dtype)
            out_avals.append(jax.core.ShapedArray(shape, dtype))
            zero_shapes.append((shape, dtype))
    n_params = len(in_names)
    n_outs = len(out_avals)
    all_in = tuple(in_names + out_names
                   + ([partition_name] if partition_name else []))
    donate = tuple(range(n_params, n_params + n_outs))

    def _body(*args):
        operands = list(args)
        if partition_name is not None:
            operands.append(bass2jax.partition_id_tensor())
        outs = bass2jax._bass_exec_p.bind(
            *operands,
            out_avals=tuple(out_avals),
            in_names=all_in,
            out_names=tuple(out_names),
            lowering_input_output_aliases=(),
            sim_require_finite=True,
            sim_require_nnan=True,
            nc=nc,
        )
        return tuple(outs)

    devices = jax.devices()[:8]
    mesh = Mesh(np.asarray(devices), ("core",))
    in_specs = (PartitionSpec("core"),) * (n_params + n_outs)
    out_specs = (PartitionSpec("core"),) * n_outs
    sharded = jax.jit(
        shard_map(_body, mesh=mesh, in_specs=in_specs, out_specs=out_specs,
                  check_rep=False),
        donate_argnums=donate,
        keep_unused=True,
    )
    runner = dict(nc=nc, sharded=sharded, in_names=in_names,
                  out_names=out_names, zero_shapes=zero_shapes, mesh=mesh)
    _CACHE[key] = runner
    return runner


def _shard_inputs(x, Wq, Wk, Wv, Wo):
    x = np.ascontiguousarray(np.asarray(x, dtype=np.float32))
    Wq = np.ascontiguousarray(np.asarray(Wq, dtype=np.float32))
    Wk = np.ascontiguousarray(np.asarray(Wk, dtype=np.float32))
    Wv = np.ascontiguousarray(np.asarray(Wv, dtype=np.float32))
    Wo = np.ascontiguousarray(np.asarray(Wo, dtype=np.float32))
    per_core = {"x": [], "Wq": [], "Wk": [], "Wv": [], "Wo": []}
    for core in range(8):
        b, h = core // 2, core % 2
        per_core["x"].append(x[b])
        per_core["Wq"].append(Wq[h * DSH:(h + 1) * DSH])
        per_core["Wk"].append(Wk[h * DSH:(h + 1) * DSH])
        per_core["Wv"].append(Wv[h * DSH:(h + 1) * DSH])
        per_core["Wo"].append(np.ascontiguousarray(Wo[:, h * DSH:(h + 1) * DSH]))
    return {k: np.concatenate(v, axis=0) for k, v in per_core.items()}


def _run(concat, runner):
    concat_in = [concat[nm] for nm in runner["in_names"]]
    concat_zeros = [np.zeros((8 * s[0], *s[1:]), d)
                    for (s, d) in runner["zero_shapes"]]
    outs = runner["sharded"](*concat_in, *concat_zeros)
    return np.asarray(outs[runner["out_names"].index("y")])


def kernel(x, Wq, Wk, Wv, Wo):
    runner = _get_runner()
    concat = _shard_inputs(x, Wq, Wk, Wv, Wo)
    y8 = _run(concat, runner).reshape(8, T, C)
    y = np.empty((B, T, C), dtype=np.float32)
    for b in range(B):
        y[b] = y8[2 * b] + y8[2 * b + 1]
    return y


def bench_hw(x, Wq, Wk, Wv, Wo, k_lo=1, k_hi=17, rounds=5, iters=2):
    """Per-run HW time via repeated-body programs: (t(k_hi)-t(k_lo))/(k_hi-k_lo).

    Alternates k_lo/k_hi measurements to cancel the (drifting) dispatch floor.
    """
    import time
    import jax
    from jax.sharding import NamedSharding, PartitionSpec

    concat = _shard_inputs(x, Wq, Wk, Wv, Wo)
    state = {}

    def prep(k):
        runner = _get_runner(reps=k)
        sh = NamedSharding(runner["mesh"], PartitionSpec("core"))
        dev_in = [jax.device_put(concat[nm], sh) for nm in runner["in_names"]]
        jax.block_until_ready(dev_in)
        zeros_np = [np.zeros((8 * s[0], *s[1:]), d)
                    for (s, d) in runner["zero_shapes"]]
        state[k] = (runner, sh, dev_in, zeros_np)

    def run_once(k):
        runner, sh, dev_in, zeros_np = state[k]
        dz = [jax.device_put(z, sh) for z in zeros_np]
        jax.block_until_ready(dz)
        t0 = time.perf_counter()
        outs = runner["sharded"](*dev_in, *dz)
        jax.block_until_ready(outs)
        return time.perf_counter() - t0, outs

    prep(k_lo)
    prep(k_hi)
    run_once(k_lo)  # warmups (compile)
    run_once(k_hi)

    t_lo, t_hi = [], []
    outs = None
    for _ in range(rounds):
        for _ in range(iters):
            t, _ = run_once(k_lo)
            t_lo.append(t)
        for _ in range(iters):
            t, outs = run_once(k_hi)
            t_hi.append(t)
    t_lo_m, t_hi_m = min(t_lo), min(t_hi)
    per_run = (t_hi_m - t_lo_m) / (k_hi - k_lo)

    runner = state[k_hi][0]
    y8 = np.asarray(outs[runner["out_names"].index("y")]).reshape(8, T, C)
    y = np.empty((B, T, C), dtype=np.float32)
    for b in range(B):
        y[b] = y8[2 * b] + y8[2 * b + 1]
    return per_run, t_lo_m, t_hi_m, y


def bench(x, Wq, Wk, Wv, Wo, iters=5):
    """Timed runs with device-resident inputs; returns (best_seconds, y)."""
    import time
    import jax
    from jax.sharding import NamedSharding, PartitionSpec

    runner = _get_runner()
    concat = _shard_inputs(x, Wq, Wk, Wv, Wo)
    sh = NamedSharding(runner["mesh"], PartitionSpec("core"))
    dev_in = [jax.device_put(concat[nm], sh) for nm in runner["in_names"]]
    jax.block_until_ready(dev_in)
    zeros_np = [np.zeros((8 * s[0], *s[1:]), d) for (s, d) in runner["zero_shapes"]]

    times = []
    outs = None
    for _ in range(iters + 1):  # first is warmup/compile
        dz = [jax.device_put(z, sh) for z in zeros_np]
        jax.block_until_ready(dz)
        t0 = time.perf_counter()
        outs = runner["sharded"](*dev_in, *dz)
        jax.block_until_ready(outs)
        times.append(time.perf_counter() - t0)
    y8 = np.asarray(outs[runner["out_names"].index("y")]).reshape(8, T, C)
    y = np.empty((B, T, C), dtype=np.float32)
    for b in range(B):
        y[b] = y8[2 * b] + y8[2 * b + 1]
    return min(times[1:]), y
